# revision 15
# baseline (speedup 1.0000x reference)
import sys

sys.path.insert(0, "/opt/trn_rl_repo")

import numpy as np
import ml_dtypes

import concourse.bass as bass
import concourse.bass_isa as bass_isa
from concourse import bacc
import concourse.mybir as mybir
import concourse.tile as tile
from concourse.bass_utils import run_bass_kernel_spmd

# Problem constants (hardcoded; see nn_ConvLSTMAutoencoder spec)
B_TOT, T, F = 128, 100, 64
NCORES = 8
B = B_TOT // NCORES          # 16 batch per core (pure data parallelism)
SEG = F + 2                  # 66: spatial row stored with 1 zero pad each side
C0, C1 = 16, 32              # encoder hidden dims; decoder mirrors [32, 16]
NQ = (T + 2) // 3            # seq: 3 slots (bases 0/32/64) per column

F32 = mybir.dt.float32
BF16 = mybir.dt.bfloat16
NP_BF16 = ml_dtypes.bfloat16

Tanh = mybir.ActivationFunctionType.Tanh
MULT = mybir.AluOpType.mult
ADD = mybir.AluOpType.add

# Hardware rules (TRN2, probed via birverifier + perfetto): every
# compute-engine operand must start at partition 0/32/64/96; spans may not
# cross upward into a lower-numbered quadrant boundary (base 0: any count,
# base 64: up to 64, bases 32/96: up to 32). Both matmul operands must share
# the same partition base (any of the four). One-input ops (activation /
# tensor_scalar) may re-base, and are fast at any base. A 2-input DVE op
# whose SOURCES sit at base 32/96 runs ~12x slower; keep those at 0/64.
# gpsimd tensor ops cost ~15us each - never use them. DMA is unconstrained
# but a DMA writing a tile serializes against all compute reads of it, so
# per-step DMA targets must not be matmul-rhs tiles (that WAR lands on the
# recurrence spine).
#
# Layout:
#  arena_e rows: 0:16 h_enc0 | 16:32 zero | 32:64 h_enc1. x lives in its own
#    double-buffered tile; its conv contribution is PSUM-accumulated early by
#    separate K=1 matmuls, so no DMA ever writes arena_e.
#  arena_d rows: 0:32 h_dec0 | 32:48 h_dec1 | 48 ones (carries fc bias).
#    The decoder input e2(t) is read straight out of the padded seq buffer
#    (no per-step DMA); its weight rows are replicated at all 4 partition
#    bases so lhsT can match the cycling seq slot base.
#  seq: [128, NQ, B, SEG] padded (col 0 and 65 stay zero) so conv taps can
#    slide over it directly.
#
# Gate layout along M for each conv matmul, gates i/f/o pre-halved so one
# tanh serves all four (sigmoid(x) = 0.5*tanh(x/2)+0.5):
#   C=32: [i 0:32 | f 32:64 | o 64:96 | g 96:128]            M = 128
#   C=16: [i 0:16 |-| f 32:48 |-| o 64:80 |-| g 96:112]      M = 112
# Per cell (ACT = scalar engine, TS/TT = vector engine, all bf16):
#   th[0:M]      = Tanh(z + b)         ACT
#   s[0:64+C]    = 0.5*th+0.5          TS   (sig_i@0, sig_o@64)
#   sf[0:C]      = 0.5*th[32:+C]+0.5   TS   (re-base 32->0)
#   gh[0:C]      = th[96:+C]           TS   (re-base 96->0)
#   u = s[0:C]*gh; v = sf*c; c = u+v   TT   (base 0)
#   tc[64:+C]    = Tanh(c)             ACT  (re-base 0->64)
#   h = s[64:+C]*tc[64:+C]             TT   (base 64)

MM_NB = 8        # batches per matmul instruction (8*64 = 512 <= psum bank)


def _mm_taps(nc, zt, wt, rhs_src, start, stop):
    """3-tap conv contribution as PSUM-accumulated matmuls (bf16)."""
    for d in range(3):
        rhs = rhs_src(d)
        for nb in range(0, B, MM_NB):
            nc.tensor.matmul(
                zt[:, nb:nb + MM_NB, :],
                wt[:, d, :],
                rhs[:, nb:nb + MM_NB, :],
                start=(start and d == 0),
                stop=(stop and d == 2),
                skip_group_check=True,
            )


def _lstm_cell(nc, wpool, z, C, M, bvec, ctile, h_out):
    """Gate math for one ConvLSTM step under the alignment + perf rules."""
    th = wpool.tile([M, B, F], BF16, tag="th", name="th")
    nc.scalar.activation(th[0:64 + C], z[0:64 + C], Tanh, bias=bvec[0:64 + C])
    gh = wpool.tile([C, B, F], BF16, tag="gh", name="gh")
    nc.scalar.activation(gh[:], z[96:96 + C], Tanh, bias=bvec[96:96 + C])
    s = wpool.tile([64 + C, B, F], BF16, tag="s", name="s")
    nc.vector.tensor_scalar(s[:], th[0:64 + C], 0.5, 0.5, MULT, ADD)
    sf = wpool.tile([C, B, F], BF16, tag="sf", name="sf")
    nc.vector.tensor_scalar(sf[:], th[32:32 + C], 0.5, 0.5, MULT, ADD)
    v = wpool.tile([C, B, F], BF16, tag="v", name="v")
    nc.vector.tensor_tensor(v[:], sf[:], ctile[:], MULT)      # sig(f)*c
    u = wpool.tile([C, B, F], BF16, tag="u", name="u")
    nc.vector.tensor_tensor(u[:], s[0:C], gh[:], MULT)        # sig(i)*tanh(g)
    nc.vector.tensor_tensor(ctile[:], u[:], v[:], ADD)        # c updated
    tc_ = wpool.tile([64 + C, B, F], BF16, tag="tc", name="tc")
    nc.scalar.activation(tc_[64:64 + C], ctile[:], Tanh)
    nc.vector.tensor_tensor(h_out, s[64:64 + C], tc_[64:64 + C], MULT)


def build_program():
    nc = bacc.Bacc(None)

    x_pad = nc.declare_dram_parameter("x_pad", [T, 3, B, F], BF16, isOutput=False)
    w0h = nc.declare_dram_parameter("w0h", [C0, 3, 112], BF16, isOutput=False)
    w0x = nc.declare_dram_parameter("w0x", [3, 112], BF16, isOutput=False)
    w1 = nc.declare_dram_parameter("w1", [2 * C1, 3, 128], BF16, isOutput=False)
    wd0e = nc.declare_dram_parameter("wd0e", [96, 3, 128], BF16, isOutput=False)
    wd0h = nc.declare_dram_parameter("wd0h", [C1, 3, 128], BF16, isOutput=False)
    wd1 = nc.declare_dram_parameter("wd1", [C1 + C0 + 1, 3, 113], BF16, isOutput=False)
    fcv = nc.declare_dram_parameter("fcv", [C0 + 1, 1], BF16, isOutput=False)
    b0 = nc.declare_dram_parameter("b0", [112, 1], F32, isOutput=False)
    b1 = nc.declare_dram_parameter("b1", [128, 1], F32, isOutput=False)
    bd0 = nc.declare_dram_parameter("bd0", [128, 1], F32, isOutput=False)
    bd1 = nc.declare_dram_parameter("bd1", [112, 1], F32, isOutput=False)
    out = nc.declare_dram_parameter("out", [B, T, F], F32, isOutput=True)

    with tile.TileContext(nc) as tc:
        with (
            tc.tile_pool(name="const", bufs=1) as cpool,
            tc.tile_pool(name="state", bufs=1) as spool,
            tc.tile_pool(name="work", bufs=2) as wpool,
            tc.tile_pool(name="zp", bufs=1, space="PSUM") as zpool,
            tc.tile_pool(name="fcp", bufs=2, space="PSUM") as fcpool,
        ):
            w0ht = cpool.tile([C0, 3, 112], BF16)
            w0xt = cpool.tile([3, 112], BF16)
            w1t = cpool.tile([2 * C1, 3, 128], BF16)
            wd0et = cpool.tile([96, 3, 128], BF16)
            wd0ht = cpool.tile([C1, 3, 128], BF16)
            wd1t = cpool.tile([C1 + C0 + 1, 3, 113], BF16)
            # fc weights staged at partitions 32:49 to match arena_d rows
            fcvt = cpool.tile([32 + C0 + 1, 1], BF16)
            b0t = cpool.tile([112, 1], F32)
            b1t = cpool.tile([128, 1], F32)
            bd0t = cpool.tile([128, 1], F32)
            bd1t = cpool.tile([112, 1], F32)
            for dst, dsrc in [(w0ht[:], w0h), (w0xt[:], w0x), (w1t[:], w1),
                              (wd0et[:], wd0e), (wd0ht[:], wd0h),
                              (wd1t[:], wd1), (fcvt[32:32 + C0 + 1], fcv),
                              (b0t[:], b0), (b1t[:], b1), (bd0t[:], bd0),
                              (bd1t[:], bd1)]:
                nc.sync.dma_start(dst, dsrc[:])

            arena_e = spool.tile([2 * C1, B, SEG], BF16)
            arena_d = spool.tile([C1 + C0 + 1, B, SEG], BF16)
            seq = spool.tile([96, NQ, B, SEG], BF16)
            nc.vector.memset(arena_e[:], 0.0)
            nc.vector.memset(arena_d[0:C1 + C0 + 1], 1.0)  # row 48 stays 1.0
            nc.vector.memset(arena_d[0:C1 + C0], 0.0)
            nc.vector.memset(seq[:], 0.0)

            # cell states at partition base 0 of their own tiles
            ce0 = spool.tile([C0, B, F], BF16)
            ce1 = spool.tile([C1, B, F], BF16)
            cd0 = spool.tile([C1, B, F], BF16)
            cd1 = spool.tile([C0, B, F], BF16)
            nc.vector.memset(ce0[:], 0.0)
            nc.vector.memset(ce1[:], 0.0)

            # ---------------- encoder ----------------
            # x(t)'s conv contribution: host pre-shifts x into 3 tap rows so
            # one K=3 matmul group (2 MMs) seeds the z0 PSUM accumulation.
            # It is issued early so the in-order tensor engine runs it during
            # cell0(t-1); z0h(t+1) is issued BEFORE z1(t) because cell0(t+1)
            # only needs z0h -- this keeps the recurrence spine at 6 MMs.
            def _xpart(t):
                xr = wpool.tile([3, B, F], BF16, tag="xr", name="xr")
                nc.sync.dma_start(xr[:], x_pad[t])
                z0n = zpool.tile([112, B, F], F32, tag="z112", name="z0")
                for nb in range(0, B, MM_NB):
                    nc.tensor.matmul(z0n[:, nb:nb + MM_NB, :], w0xt[:],
                                     xr[:, nb:nb + MM_NB, :],
                                     start=True, stop=False,
                                     skip_group_check=True)
                return z0n

            def _z0h(z0n):
                _mm_taps(nc, z0n, w0ht, lambda d: arena_e[0:C0, :, d:d + F],
                         start=False, stop=True)

            def _cell0(z):
                _lstm_cell(nc, wpool, z, C0, 112, b0t, ce0,
                           arena_e[0:C0, :, 1:1 + F])

            def _enc1(t):
                z1 = zpool.tile([128, B, F], F32, tag="z128", name="z1")
                _mm_taps(nc, z1, w1t, lambda d: arena_e[0:2 * C1, :, d:d + F],
                         start=True, stop=True)
                return z1

            def _cell1_seq(t, z1):
                _lstm_cell(nc, wpool, z1, C1, 128, b1t, ce1,
                           arena_e[C1:2 * C1, :, 1:1 + F])
                r = (t % 3) * 32
                nc.sync.dma_start(seq[r:r + 32, t // 3, :, 1:1 + F],
                                  arena_e[C1:2 * C1, :, 1:1 + F])

            # Layer 0 runs one timestep ahead of layer 1 so each in-order
            # engine queue receives ops in readiness order: cell0(t+1)'s
            # scalar/DVE ops are issued before cell1(t)'s (which wait on
            # z1(t)), and z0h(t+1) precedes z1(t) on the tensor queue.
            z0 = _xpart(0)
            _z0h(z0)
            _cell0(z0)
            z0 = _xpart(1)
            _z0h(z0)
            for t in range(T - 1):
                z1 = _enc1(t)
                _cell0(z0)
                if t + 2 < T:
                    z0 = _xpart(t + 2)
                    _z0h(z0)
                _cell1_seq(t, z1)
            z1 = _enc1(T - 1)
            _cell1_seq(T - 1, z1)

            # ---------------- decoder init ----------------
            nc.sync.dma_start(arena_d[0:C1, :, :], arena_e[C1:2 * C1, :, :])
            nc.sync.dma_start(arena_d[C1:C1 + C0, :, :], arena_e[0:C0, :, :])
            nc.sync.dma_start(cd0[:], ce1[:])
            nc.sync.dma_start(cd1[:], ce0[:])

            # ---------------- decoder ----------------
            # e2-part matmuls for step t+1 are issued early (their seq data
            # has been ready since the encoder phase); the fc for step t-1 is
            # issued before cell_d1(t) overwrites h_d1, so fc never blocks
            # the next zd0 block on the in-order tensor engine.
            def _e2part(t):
                r = (t % 3) * 32
                zd0n = zpool.tile([128, B, F], F32, tag="z128", name="zd0")
                _mm_taps(nc, zd0n, wd0et[r:r + 32],
                         lambda d: seq[r:r + 32, t // 3, :, d:d + F],
                         start=True, stop=False)
                return zd0n

            def _fc(t):
                # final 1x1 conv: K=17, M=1 matmul read at partition base 32
                # (ones row carries fc_b), then DVE copy out of PSUM.
                zfc = fcpool.tile([1, B, F], F32, tag="fc", name="zfc")
                for nb in range(0, B, MM_NB):
                    nc.tensor.matmul(
                        zfc[:, nb:nb + MM_NB, :],
                        fcvt[32:32 + C0 + 1],
                        arena_d[C1:C1 + C0 + 1, nb:nb + MM_NB, 1:1 + F],
                        start=True, stop=True)
                ofc = wpool.tile([1, B, F], F32, tag="ofc", name="ofc")
                nc.vector.tensor_scalar_mul(ofc[:], zfc[:], 1.0)
                nc.sync.dma_start(out[:, t, :], ofc[0:1, :, :])

            def _zd0h(zd0n):
                _mm_taps(nc, zd0n, wd0ht,
                         lambda d: arena_d[0:C1, :, d:d + F],
                         start=False, stop=True)

            def _celld0(z):
                _lstm_cell(nc, wpool, z, C1, 128, bd0t, cd0,
                           arena_d[0:C1, :, 1:1 + F])

            def _dec1(t):
                # col 112 of wd1 carries the fc weights for h_d1(t-1) (plus
                # fc_b on the ones row), so fc(t-1) rides along in this
                # matmul group for free.
                zd1 = zpool.tile([113, B, F], F32, tag="z112", name="zd1")
                _mm_taps(nc, zd1, wd1t,
                         lambda d: arena_d[0:C1 + C0 + 1, :, d:d + F],
                         start=True, stop=True)
                if t > 0:
                    ofc = wpool.tile([17, B, F], F32, tag="ofc", name="ofc")
                    nc.vector.tensor_scalar_mul(ofc[:], zd1[96:96 + 17], 1.0)
                    nc.sync.dma_start(out[:, t - 1, :], ofc[16:17, :, :])
                return zd1

            def _celld1(z):
                _lstm_cell(nc, wpool, z, C0, 112, bd1t, cd1,
                           arena_d[C1:C1 + C0, :, 1:1 + F])

            # Same software pipelining as the encoder: layer d0 runs one
            # step ahead of d1; fc(t-1) is issued before cell_d1(t)
            # overwrites h_d1 and never blocks the spine.
            zd0 = _e2part(0)
            _zd0h(zd0)
            _celld0(zd0)
            zd0 = _e2part(1)
            _zd0h(zd0)
            for t in range(T - 1):
                zd1 = _dec1(t)
                _celld0(zd0)
                if t + 2 < T:
                    zd0 = _e2part(t + 2)
                    _zd0h(zd0)
                _celld1(zd1)
            zd1 = _dec1(T - 1)
            _celld1(zd1)
            _fc(T - 1)

    nc.finalize()
    return nc


# M-column spread per gate, by hidden size
def _m_cols(C):
    return {"i": 0, "f": 32, "o": 64, "g": 96}, 96 + C


def _prep_weights(w, b, Cin, C, row_order):
    """[4C, Cin, 3, 3] -> lhsT [len(row_order), 3, M] with the gate spread.

    Reference gate order along output channels is i, f, o, g. The i/f/o rows
    (and biases) are halved so one tanh serves all gates. row_order maps
    lhsT row -> input channel (-1 = zero row).
    """
    cols, M = _m_cols(C)
    w3 = np.asarray(w, np.float32).reshape(4 * C, Cin, 3, 3)[:, :, :, 1]
    b = np.asarray(b, np.float32).reshape(4 * C)
    gate_of = {"i": 0, "f": 1, "o": 2, "g": 3}
    lhsT = np.zeros((len(row_order), 3, M), np.float32)
    bvec = np.zeros((M, 1), np.float32)
    for gname, col0 in cols.items():
        gi = gate_of[gname]
        scale = 0.5 if gname in ("i", "f", "o") else 1.0
        for j in range(C):
            oc = gi * C + j
            bvec[col0 + j, 0] = b[oc] * scale
            for r, ch in enumerate(row_order):
                if ch >= 0:
                    lhsT[r, :, col0 + j] = w3[oc, ch, :] * scale
    return np.ascontiguousarray(lhsT).astype(NP_BF16), bvec


_CACHE = {}


def kernel(x, enc_w0, enc_b0, enc_w1, enc_b1, dec_w0, dec_b0, dec_w1, dec_b1,
           fc_w, fc_b):
    if "nc" not in _CACHE:
        _CACHE["nc"] = build_program()
    nc = _CACHE["nc"]

    x = np.asarray(x, np.float32)
    # enc0: channel 0 = x (own tile), channels 1..16 = h0 (arena rows 0:16)
    w0full, b0 = _prep_weights(enc_w0, enc_b0, 1 + C0, C0,
                               row_order=[0] + list(range(1, 17)))
    w0x = np.ascontiguousarray(w0full[0])       # [3, 112] tap-stacked
    w0h = np.ascontiguousarray(w0full[1:17])
    # enc1 rhs rows 0:64: h0 (ch 0..15), 16 zero rows, h1 (ch 16..47)
    w1, b1 = _prep_weights(enc_w1, enc_b1, C0 + C1, C1,
                           row_order=list(range(16)) + [-1] * 16 +
                           list(range(16, 48)))
    # dec0: e2 part (ch 0..31) replicated at all 4 partition bases;
    # h_d0 part (ch 32..63) at base 0
    wd0full, bd0 = _prep_weights(dec_w0, dec_b0, C1 + C1, C1,
                                 row_order=list(range(64)))
    wd0e = np.ascontiguousarray(np.tile(wd0full[0:32], (3, 1, 1)))
    wd0h = np.ascontiguousarray(wd0full[32:64])
    # dec1 rhs rows 0:49: h_d0 (ch 0..31), h_d1 (ch 32..47), ones row 48.
    # Col 112 = fc output: fc_w on the h_d1 rows (center tap), fc_b on row 48.
    wd1g, bd1 = _prep_weights(dec_w1, dec_b1, C1 + C0, C0,
                              row_order=list(range(48)))
    wd1 = np.zeros((49, 3, 113), np.float32)
    wd1[0:48, :, 0:112] = np.asarray(wd1g, np.float32)
    wd1[32:48, 1, 112] = np.asarray(fc_w, np.float32).reshape(C0)
    wd1[48, 1, 112] = np.asarray(fc_b, np.float32).reshape(1)[0]
    wd1 = np.ascontiguousarray(wd1).astype(NP_BF16)
    fcv = np.concatenate(
        [np.asarray(fc_w, np.float32).reshape(C0),
         np.asarray(fc_b, np.float32).reshape(1)]).reshape(C0 + 1, 1)
    fcv = np.ascontiguousarray(fcv.astype(NP_BF16))

    in_maps = []
    for core in range(NCORES):
        xs = x[core * B:(core + 1) * B]      # [B, T, F]
        xp = np.zeros((T, B, SEG), np.float32)
        xp[:, :, 1:1 + F] = xs.transpose(1, 0, 2)
        # tap-stacked: xp3[t, d, b, c] = xp[t, b, c + d]
        xp3 = np.stack([xp[:, :, d:d + F] for d in range(3)], axis=1)
        in_maps.append({
            "x_pad": np.ascontiguousarray(xp3).astype(NP_BF16),
            "w0h": w0h, "w0x": w0x, "w1": w1,
            "wd0e": wd0e, "wd0h": wd0h, "wd1": wd1, "fcv": fcv,
            "b0": b0, "b1": b1, "bd0": bd0, "bd1": bd1,
        })

    _CACHE["in_maps"] = in_maps
    res = run_bass_kernel_spmd(nc, in_maps, core_ids=list(range(NCORES)))
    outs = [res.results[i]["out"] for i in range(NCORES)]
    return np.concatenate(outs, axis=0).astype(np.float32)


if __name__ == "__main__":
    rng = np.random.default_rng(0)
    inputs = {
        "x": rng.standard_normal((B_TOT, T, F), dtype=np.float32),
        "enc_w0": rng.standard_normal((4 * C0, 1 + C0, 3, 3), dtype=np.float32) * 0.05,
        "enc_b0": np.zeros(4 * C0, np.float32),
        "enc_w1": rng.standard_normal((4 * C1, C0 + C1, 3, 3), dtype=np.float32) * 0.05,
        "enc_b1": np.zeros(4 * C1, np.float32),
        "dec_w0": rng.standard_normal((4 * C1, C1 + C1, 3, 3), dtype=np.float32) * 0.05,
        "dec_b0": np.zeros(4 * C1, np.float32),
        "dec_w1": rng.standard_normal((4 * C0, C1 + C0, 3, 3), dtype=np.float32) * 0.05,
        "dec_b1": np.zeros(4 * C0, np.float32),
        "fc_w": rng.standard_normal((1, C0, 1, 1), dtype=np.float32) * 0.05,
        "fc_b": np.zeros(1, np.float32),
    }
    out = kernel(**inputs)
    print("out", out.shape, out.dtype, np.abs(out).max())
